# revision 2
# baseline (speedup 1.0000x reference)
"""Distributed kNN codebook kernel for Trainium2 (8 NeuronCores).

Sharding: codebook dimension N=50000 split 8 ways (6250 rows/core, padded to
6272).  Each core computes [2048, 6272] similarity panels on the PE, does a
local top-24 per query row on the DVE (max8 / max_index / match_replace), and
writes 24 candidates/row.  Host merges 8*24 candidates to the global top-20
and gathers labels.

Selection math (order-preserving per row, exact values reconstructed on host):
  cos:    raw dot z_rot . book      (renorm of z is a positive row scale)
  euclid: s = z_trans . book - |book|^2/2   (m2 = 2*s - |z|^2)
"""

import sys

sys.path.insert(0, "/opt/trn_rl_repo")

import numpy as np

import concourse.bass as bass
import concourse.bacc as bacc
import concourse.mybir as mybir
from concourse.bass import ts
from concourse.masks import make_identity
from concourse.tile import TileContext
from concourse.bass_utils import run_bass_kernel_spmd

B = 2048
N = 50000
D = 128
K = 20
NCORES = 8
NL = N // NCORES            # 6250 codebook rows per core
NCHUNK = (NL + 127) // 128  # 49 transpose chunks
NP = NCHUNK * 128           # 6272 padded width
NCAND = 24                  # local candidates per row (3 max8 rounds)
NEG = -1.0e30

F32 = mybir.dt.float32
U32 = mybir.dt.uint32

# panel split of the 6272-wide similarity row: 12 x 512 + 1 x 128
PANELS = [(i * 512, 512) for i in range(12)] + [(12 * 512, 128)]


def _topk_rounds(nc, x, mvals, midx, nrounds=3):
    """Top (8*nrounds) values+indices per partition row of x, descending.
    x is destroyed. mvals/midx are [128, 8*nrounds] f32/u32 tiles."""
    for r in range(nrounds):
        mv = mvals[:, 8 * r : 8 * r + 8]
        nc.vector.max(out=mv, in_=x)
        nc.vector.max_index(out=midx[:, 8 * r : 8 * r + 8], in_max=mv, in_values=x)
        if r + 1 < nrounds:
            nc.vector.match_replace(out=x, in_to_replace=mv, in_values=x, imm_value=NEG)


def build_program():
    nc = bacc.Bacc("TRN2", target_bir_lowering=False, debug=False, num_devices=NCORES)

    zr = nc.dram_tensor("zr", [B, D], F32, kind="ExternalInput")
    zt = nc.dram_tensor("zt", [B, D], F32, kind="ExternalInput")
    rb = nc.dram_tensor("rb", [NP, D], F32, kind="ExternalInput")  # rot book slice
    tb = nc.dram_tensor("tb", [NP, D], F32, kind="ExternalInput")  # trans book slice

    cv = nc.dram_tensor("cv", [B, NCAND], F32, kind="ExternalOutput")
    ci = nc.dram_tensor("ci", [B, NCAND], U32, kind="ExternalOutput")
    ev = nc.dram_tensor("ev", [B, NCAND], F32, kind="ExternalOutput")
    ei = nc.dram_tensor("ei", [B, NCAND], U32, kind="ExternalOutput")

    bq_dram = nc.dram_tensor("bq_dram", [NP], F32)  # internal: -|x|^2/2 per entry

    with TileContext(nc) as tc:
        with (
            tc.tile_pool(name="persist", bufs=1) as persist,
            tc.tile_pool(name="ld", bufs=3) as ldpool,
            tc.tile_pool(name="tpsum", bufs=2, space="PSUM") as tpsum,
            tc.tile_pool(name="mmpsum", bufs=4, space="PSUM") as mmpsum,
            tc.tile_pool(name="s", bufs=2) as spool,
            tc.tile_pool(name="cand", bufs=2) as candpool,
            tc.tile_pool(name="small", bufs=2) as small,
        ):
            ident = persist.tile([128, 128], F32)
            make_identity(nc, ident)

            # ---- transpose queries: zT[d, b] for both query sets ----
            zrT = persist.tile([128, B], F32, tag="zrT")
            ztT = persist.tile([128, B], F32, tag="ztT")
            for src, dstT in ((zr, zrT), (zt, ztT)):
                for c in range(B // 128):
                    lt = ldpool.tile([128, D], F32, tag="qld")
                    nc.sync.dma_start(lt[:], src[ts(c, 128), :])
                    pt = tpsum.tile([128, 128], F32, tag="tp")
                    nc.tensor.transpose(pt, lt, ident)
                    nc.scalar.copy(dstT[:, ts(c, 128)], pt)

            # ---- transpose books: bookT[d, n]; trans book also -> bq ----
            rbT = persist.tile([128, NP], F32, tag="rbT")
            tbT = persist.tile([128, NP], F32, tag="tbT")
            bqcol = persist.tile([128, NCHUNK], F32, tag="bqcol")
            sqscr = persist.tile([128, D], F32, tag="sqscr")
            for src, dstT, want_bq in ((rb, rbT, False), (tb, tbT, True)):
                for c in range(NCHUNK):
                    lt = ldpool.tile([128, D], F32, tag="bld")
                    nc.sync.dma_start(lt[:], src[ts(c, 128), :])
                    if want_bq:
                        nc.scalar.activation(
                            sqscr[:],
                            lt[:],
                            mybir.ActivationFunctionType.Square,
                            accum_out=bqcol[:, c : c + 1],
                        )
                    pt = tpsum.tile([128, 128], F32, tag="tp")
                    nc.tensor.transpose(pt, lt, ident)
                    nc.scalar.copy(dstT[:, ts(c, 128)], pt)

            # bq -> -bq/2, reshape [128, 49] -> flat [6272] via DRAM, pad = NEG
            nc.scalar.mul(bqcol[:], bqcol[:], -0.5)
            bq_view = bq_dram.ap().rearrange("(c p) -> p c", p=128)  # [128, 49]
            nc.sync.dma_start(bq_view, bqcol[:])
            nbq = persist.tile([1, NP], F32, tag="nbq")
            nc.sync.dma_start(nbq[:], bq_dram.ap().rearrange("(a n) -> a n", a=1))
            if NP > NL:
                nc.vector.memset(nbq[:, NL:NP], NEG)

            ones1 = persist.tile([1, 128], F32, tag="ones1")
            nc.vector.memset(ones1[:], 1.0)

            # ---- main: per book, per 128-row query tile ----
            for book, (qT, bT) in enumerate(((zrT, rbT), (ztT, tbT))):
                for bt in range(B // 128):
                    s = spool.tile([128, NP], F32, tag="s")
                    for p0, pw in PANELS:
                        ps = mmpsum.tile([128, 512], F32, tag="mm")
                        nc.tensor.matmul(
                            ps[:, :pw],
                            qT[:, ts(bt, 128)],
                            bT[:, p0 : p0 + pw],
                            start=True,
                            stop=(book == 0),
                        )
                        if book == 1:
                            nc.tensor.matmul(
                                ps[:, :pw],
                                ones1[:],
                                nbq[:, p0 : p0 + pw],
                                start=False,
                                stop=True,
                            )
                        nc.scalar.copy(s[:, p0 : p0 + pw], ps[:, :pw])
                    if book == 0 and NP > NL:
                        nc.vector.memset(s[:, NL:NP], NEG)

                    mvals = candpool.tile([128, NCAND], F32, tag="mv")
                    midx = candpool.tile([128, NCAND], U32, tag="mi")
                    _topk_rounds(nc, s, mvals, midx)
                    vdst, idst = ((cv, ci), (ev, ei))[book]
                    nc.sync.dma_start(vdst[ts(bt, 128), :], mvals[:])
                    nc.sync.dma_start(idst[ts(bt, 128), :], midx[:])

    nc.compile()
    return nc


_NC = None


def _get_program():
    global _NC
    if _NC is None:
        _NC = build_program()
    return _NC


def kernel(z_rot, z_trans, z_rot_book, z_trans_book, rot_book, trans_book):
    z_rot = np.ascontiguousarray(z_rot, dtype=np.float32)
    z_trans = np.ascontiguousarray(z_trans, dtype=np.float32)
    z_rot_book = np.asarray(z_rot_book, dtype=np.float32)
    z_trans_book = np.asarray(z_trans_book, dtype=np.float32)

    nc = _get_program()

    in_maps = []
    for c in range(NCORES):
        sl = slice(c * NL, (c + 1) * NL)
        rbp = np.zeros((NP, D), np.float32)
        rbp[:NL] = z_rot_book[sl]
        tbp = np.zeros((NP, D), np.float32)
        tbp[:NL] = z_trans_book[sl]
        in_maps.append({"zr": z_rot, "zt": z_trans, "rb": rbp, "tb": tbp})

    global _last_in_maps
    _last_in_maps = in_maps
    res = run_bass_kernel_spmd(nc, in_maps, list(range(NCORES))).results

    # ---- host: gather candidates, merge to global top-20 ----
    cv = np.stack([res[c]["cv"] for c in range(NCORES)])  # [8, B, NCAND]
    ci = np.stack([res[c]["ci"] for c in range(NCORES)]).astype(np.int64)
    ev = np.stack([res[c]["ev"] for c in range(NCORES)])
    ei = np.stack([res[c]["ei"] for c in range(NCORES)]).astype(np.int64)

    offs = (np.arange(NCORES) * NL)[:, None, None]
    ci_g = np.transpose(ci + offs, (1, 0, 2)).reshape(B, -1)
    ei_g = np.transpose(ei + offs, (1, 0, 2)).reshape(B, -1)
    cv_g = np.transpose(cv, (1, 0, 2)).reshape(B, -1)
    ev_g = np.transpose(ev, (1, 0, 2)).reshape(B, -1)

    norm_r = np.sqrt(np.einsum("bd,bd->b", z_rot, z_rot))
    zq = np.einsum("bd,bd->b", z_trans, z_trans)

    cos_vals = cv_g / norm_r[:, None]          # true cosine values
    euc_vals = 2.0 * ev_g - zq[:, None]        # true m2 values

    def merge(vals, idx):
        # top-20 per row, descending, ties -> lowest index (matches lax.top_k)
        order = np.lexsort((idx, -vals), axis=1)[:, :K]
        r = np.arange(B)[:, None]
        return vals[r, order], idx[r, order]

    vals_cos, ind_cos = merge(cos_vals, ci_g)
    vals_euc, ind_euc = merge(euc_vals, ei_g)

    # safety fallback: rows with duplicate candidate indices (exact-value ties
    # can confuse max_index) get recomputed exactly on host.  ~never triggers.
    def fix_rows(bad, book, q, qscale, vals, ind, is_cos):
        for b in np.nonzero(bad)[0]:
            s = book.astype(np.float32) @ q[b].astype(np.float32)
            if is_cos:
                s = s / qscale[b]
            else:
                s = 2.0 * s - np.einsum("nd,nd->n", book, book) - qscale[b]
            o = np.lexsort((np.arange(N), -s))[:K]
            vals[b], ind[b] = s[o], o

    bad_c = (np.sort(ci_g, axis=1)[:, 1:] == np.sort(ci_g, axis=1)[:, :-1]).any(1)
    bad_e = (np.sort(ei_g, axis=1)[:, 1:] == np.sort(ei_g, axis=1)[:, :-1]).any(1)
    if bad_c.any():
        fix_rows(bad_c, z_rot_book, z_rot, norm_r, vals_cos, ind_cos, True)
    if bad_e.any():
        fix_rows(bad_e, z_trans_book, z_trans, zq, vals_euc, ind_euc, False)

    ind_cos = ind_cos.astype(np.int32)
    ind_euc = ind_euc.astype(np.int32)
    labels_rot = np.asarray(rot_book)[ind_cos]      # [B, K, 1]
    labels_trans = np.asarray(trans_book)[ind_euc]  # [B, K, 3]

    return (
        vals_cos.astype(np.float32),
        ind_cos,
        labels_rot,
        vals_euc.astype(np.float32),
        ind_euc,
        labels_trans,
    )


# revision 5
# speedup vs baseline: 2229.8639x; 2229.8639x over previous
"""Distributed kNN codebook kernel for Trainium2 (8 NeuronCores).

Sharding: codebook dimension N=50000 split 8 ways (6250 rows/core, padded to
6272).  Each core computes [2048, 6272] similarity panels on the PE, does a
local top-24 per query row on the DVE (max8 / max_index / match_replace), and
writes 24 candidates/row.  Host merges 8*24 candidates to the global top-20
and gathers labels.

Selection math (order-preserving per row, exact values reconstructed on host):
  cos:    raw dot z_rot . book      (renorm of z is a positive row scale)
  euclid: s = z_trans . book - |book|^2/2   (m2 = 2*s - |z|^2)
"""

import sys

sys.path.insert(0, "/opt/trn_rl_repo")

import numpy as np

import concourse.bass as bass
import concourse.bacc as bacc
import concourse.mybir as mybir
from concourse.bass import ts
from concourse.masks import make_identity
from concourse.tile import TileContext
from concourse.bass_utils import run_bass_kernel_spmd

B = 2048
N = 50000
D = 128
K = 20
NCORES = 8
NL = N // NCORES            # 6250 codebook rows per core
NCHUNK = (NL + 127) // 128  # 49 transpose chunks
NP = NCHUNK * 128           # 6272 padded width
NCAND = 24                  # local candidates per row (3 max8 rounds)
NEG = -1.0e30

F32 = mybir.dt.float32
U32 = mybir.dt.uint32

# panel split of the 6272-wide similarity row: 12 x 512 + 1 x 128
PANELS = [(i * 512, 512) for i in range(12)] + [(12 * 512, 128)]


CHUNK = 256
NCH = 25  # 24 x 256 + 1 x 128 chunks of the 6272-wide row


def _topk_rounds(nc, x, mvals, midx, m1, nrounds=3):
    """Top (8*nrounds) values+indices per partition row of x [128, NP].

    Level 1: top-8 of each 256-chunk (m1 [128, 8*NCH]).  A chunk holding more
    than 8 of the row's top-24 loses the excess -- the host detects chunk
    exhaustion (8 finalists from one chunk) and recomputes those rows (~1e-4).
    Level 2: 3 max8/match_replace rounds on m1 give the top-24 values; their
    positions come from batched max_index scans of the untouched raw row.
    """
    for c in range(NCH):
        w = min(CHUNK, NP - c * CHUNK)
        nc.vector.max(out=m1[:, 8 * c : 8 * c + 8], in_=x[:, c * CHUNK : c * CHUNK + w])
    for r in range(nrounds):
        mv = mvals[:, 8 * r : 8 * r + 8]
        nc.vector.max(out=mv, in_=m1)
        if r + 1 < nrounds:
            nc.vector.match_replace(out=m1, in_to_replace=mv, in_values=m1, imm_value=NEG)
    for r in range(nrounds):
        nc.vector.max_index(
            out=midx[:, 8 * r : 8 * r + 8], in_max=mvals[:, 8 * r : 8 * r + 8], in_values=x
        )


def build_program():
    nc = bacc.Bacc("TRN2", target_bir_lowering=False, debug=False, num_devices=NCORES)

    zr = nc.dram_tensor("zr", [B, D], F32, kind="ExternalInput")
    zt = nc.dram_tensor("zt", [B, D], F32, kind="ExternalInput")
    rb = nc.dram_tensor("rb", [NP, D], F32, kind="ExternalInput")  # rot book slice
    tb = nc.dram_tensor("tb", [NP, D], F32, kind="ExternalInput")  # trans book slice

    cv = nc.dram_tensor("cv", [B, NCAND], F32, kind="ExternalOutput")
    ci = nc.dram_tensor("ci", [B, NCAND], U32, kind="ExternalOutput")
    ev = nc.dram_tensor("ev", [B, NCAND], F32, kind="ExternalOutput")
    ei = nc.dram_tensor("ei", [B, NCAND], U32, kind="ExternalOutput")

    bq_dram = nc.dram_tensor("bq_dram", [NP], F32)  # internal: -|x|^2/2 per entry

    with TileContext(nc) as tc:
        with (
            tc.tile_pool(name="persist", bufs=1) as persist,
            tc.tile_pool(name="ld", bufs=3) as ldpool,
            tc.tile_pool(name="tpsum", bufs=2, space="PSUM") as tpsum,
            tc.tile_pool(name="mmpsum", bufs=4, space="PSUM") as mmpsum,
            tc.tile_pool(name="s", bufs=2) as spool,
            tc.tile_pool(name="cand", bufs=2) as candpool,
            tc.tile_pool(name="small", bufs=2) as small,
        ):
            ident = persist.tile([128, 128], F32)
            make_identity(nc, ident)

            # ---- transpose queries: zT[d, b] for both query sets ----
            zrT = persist.tile([128, B], F32, tag="zrT")
            ztT = persist.tile([128, B], F32, tag="ztT")
            for src, dstT in ((zr, zrT), (zt, ztT)):
                for c in range(B // 128):
                    lt = ldpool.tile([128, D], F32, tag="qld")
                    nc.sync.dma_start(lt[:], src[ts(c, 128), :])
                    pt = tpsum.tile([128, 128], F32, tag="tp")
                    nc.tensor.transpose(pt, lt, ident)
                    nc.scalar.copy(dstT[:, ts(c, 128)], pt)

            # ---- transpose books: bookT[d, n]; trans book also -> bq ----
            rbT = persist.tile([128, NP], F32, tag="rbT")
            tbT = persist.tile([128, NP], F32, tag="tbT")
            bqcol = persist.tile([128, NCHUNK], F32, tag="bqcol")
            sqscr = persist.tile([128, D], F32, tag="sqscr")
            for src, dstT, want_bq in ((rb, rbT, False), (tb, tbT, True)):
                for c in range(NCHUNK):
                    lt = ldpool.tile([128, D], F32, tag="bld")
                    nc.sync.dma_start(lt[:], src[ts(c, 128), :])
                    if want_bq:
                        nc.scalar.activation(
                            sqscr[:],
                            lt[:],
                            mybir.ActivationFunctionType.Square,
                            accum_out=bqcol[:, c : c + 1],
                        )
                    pt = tpsum.tile([128, 128], F32, tag="tp")
                    nc.tensor.transpose(pt, lt, ident)
                    nc.scalar.copy(dstT[:, ts(c, 128)], pt)

            # bq -> -bq/2, reshape [128, 49] -> flat [6272] via DRAM, pad = NEG
            nc.scalar.mul(bqcol[:], bqcol[:], -0.5)
            bq_view = bq_dram.ap().rearrange("(c p) -> p c", p=128)  # [128, 49]
            nc.sync.dma_start(bq_view, bqcol[:])
            nbq = persist.tile([1, NP], F32, tag="nbq")
            nc.sync.dma_start(nbq[:], bq_dram.ap().rearrange("(a n) -> a n", a=1))
            if NP > NL:
                nc.vector.memset(nbq[:, NL:NP], NEG)

            ones1 = persist.tile([1, 128], F32, tag="ones1")
            nc.vector.memset(ones1[:], 1.0)

            # ---- main: per book, per 128-row query tile ----
            for book, (qT, bT) in enumerate(((zrT, rbT), (ztT, tbT))):
                for bt in range(B // 128):
                    s = spool.tile([128, NP], F32, tag="s")
                    for p0, pw in PANELS:
                        ps = mmpsum.tile([128, 512], F32, tag="mm")
                        nc.tensor.matmul(
                            ps[:, :pw],
                            qT[:, ts(bt, 128)],
                            bT[:, p0 : p0 + pw],
                            start=True,
                            stop=(book == 0),
                        )
                        if book == 1:
                            nc.tensor.matmul(
                                ps[:, :pw],
                                ones1[:],
                                nbq[:, p0 : p0 + pw],
                                start=False,
                                stop=True,
                            )
                        nc.scalar.copy(s[:, p0 : p0 + pw], ps[:, :pw])
                    if book == 0 and NP > NL:
                        nc.vector.memset(s[:, NL:NP], NEG)

                    mvals = candpool.tile([128, NCAND], F32, tag="mv")
                    midx = candpool.tile([128, NCAND], U32, tag="mi")
                    m1 = candpool.tile([128, 8 * NCH], F32, tag="m1")
                    _topk_rounds(nc, s, mvals, midx, m1)
                    vdst, idst = ((cv, ci), (ev, ei))[book]
                    nc.sync.dma_start(vdst[ts(bt, 128), :], mvals[:])
                    nc.sync.dma_start(idst[ts(bt, 128), :], midx[:])

    nc.compile()
    return nc


_NC = None


def _get_program():
    global _NC
    if _NC is None:
        _NC = build_program()
    return _NC


def kernel(z_rot, z_trans, z_rot_book, z_trans_book, rot_book, trans_book):
    z_rot = np.ascontiguousarray(z_rot, dtype=np.float32)
    z_trans = np.ascontiguousarray(z_trans, dtype=np.float32)
    z_rot_book = np.asarray(z_rot_book, dtype=np.float32)
    z_trans_book = np.asarray(z_trans_book, dtype=np.float32)

    nc = _get_program()

    in_maps = []
    for c in range(NCORES):
        sl = slice(c * NL, (c + 1) * NL)
        rbp = np.zeros((NP, D), np.float32)
        rbp[:NL] = z_rot_book[sl]
        tbp = np.zeros((NP, D), np.float32)
        tbp[:NL] = z_trans_book[sl]
        in_maps.append({"zr": z_rot, "zt": z_trans, "rb": rbp, "tb": tbp})

    global _last_in_maps
    _last_in_maps = in_maps
    res = run_bass_kernel_spmd(nc, in_maps, list(range(NCORES))).results

    # ---- host: gather candidates, merge to global top-20 ----
    cv = np.stack([res[c]["cv"] for c in range(NCORES)])  # [8, B, NCAND]
    ci = np.stack([res[c]["ci"] for c in range(NCORES)]).astype(np.int64)
    ev = np.stack([res[c]["ev"] for c in range(NCORES)])
    ei = np.stack([res[c]["ei"] for c in range(NCORES)]).astype(np.int64)

    offs = (np.arange(NCORES) * NL)[:, None, None]
    ci_g = np.transpose(ci + offs, (1, 0, 2)).reshape(B, -1)
    ei_g = np.transpose(ei + offs, (1, 0, 2)).reshape(B, -1)
    cv_g = np.transpose(cv, (1, 0, 2)).reshape(B, -1)
    ev_g = np.transpose(ev, (1, 0, 2)).reshape(B, -1)

    norm_r = np.sqrt(np.einsum("bd,bd->b", z_rot, z_rot))
    zq = np.einsum("bd,bd->b", z_trans, z_trans)

    cos_vals = cv_g / norm_r[:, None]          # true cosine values
    euc_vals = 2.0 * ev_g - zq[:, None]        # true m2 values

    def merge(vals, idx):
        # top-20 per row, descending, ties -> lowest index (matches lax.top_k)
        order = np.lexsort((idx, -vals), axis=1)[:, :K]
        r = np.arange(B)[:, None]
        return vals[r, order], idx[r, order]

    vals_cos, ind_cos = merge(cos_vals, ci_g)
    vals_euc, ind_euc = merge(euc_vals, ei_g)

    # safety fallback: rows with duplicate candidate indices (exact-value ties
    # can confuse max_index) get recomputed exactly on host.  ~never triggers.
    def fix_rows(bad, book, q, qscale, vals, ind, is_cos):
        for b in np.nonzero(bad)[0]:
            s = book.astype(np.float32) @ q[b].astype(np.float32)
            if is_cos:
                s = s / qscale[b]
            else:
                s = 2.0 * s - np.einsum("nd,nd->n", book, book) - qscale[b]
            o = np.lexsort((np.arange(N), -s))[:K]
            vals[b], ind[b] = s[o], o

    def suspicious(idx_g):
        # duplicate candidate indices (exact-value ties through max_index)
        srt = np.sort(idx_g, axis=1)
        bad = (srt[:, 1:] == srt[:, :-1]).any(1)
        # chunk exhaustion: 8 of one core's 24 candidates from one 256-chunk
        ch = np.sort((idx_g % NL) // CHUNK + (idx_g // NL) * 1000, axis=1)
        bad |= (ch[:, 7:] == ch[:, :-7]).any(1)
        return bad

    bad_c = suspicious(ci_g)
    bad_e = suspicious(ei_g)
    if bad_c.any():
        fix_rows(bad_c, z_rot_book, z_rot, norm_r, vals_cos, ind_cos, True)
    if bad_e.any():
        fix_rows(bad_e, z_trans_book, z_trans, zq, vals_euc, ind_euc, False)

    ind_cos = ind_cos.astype(np.int32)
    ind_euc = ind_euc.astype(np.int32)
    labels_rot = np.asarray(rot_book)[ind_cos]      # [B, K, 1]
    labels_trans = np.asarray(trans_book)[ind_euc]  # [B, K, 3]

    return (
        vals_cos.astype(np.float32),
        ind_cos,
        labels_rot,
        vals_euc.astype(np.float32),
        ind_euc,
        labels_trans,
    )


# revision 8
# speedup vs baseline: 2931.7270x; 1.3148x over previous
"""Distributed kNN codebook kernel for Trainium2 (8 NeuronCores).

Sharding: codebook dimension N=50000 split 8 ways (6250 rows/core, padded to
6272).  Each core computes [2048, 6272] similarity panels on the PE, does a
local top-24 per query row on the DVE (max8 / max_index / match_replace), and
writes 24 candidates/row.  Host merges 8*24 candidates to the global top-20
and gathers labels.

Selection math (order-preserving per row, exact values reconstructed on host):
  cos:    raw dot z_rot . book      (renorm of z is a positive row scale)
  euclid: s = z_trans . book - |book|^2/2   (m2 = 2*s - |z|^2)
"""

import sys

sys.path.insert(0, "/opt/trn_rl_repo")

import numpy as np

import concourse.bass as bass
import concourse.bacc as bacc
import concourse.mybir as mybir
from concourse.bass import ts
from concourse.masks import make_identity
from concourse.tile import TileContext
from concourse.bass_utils import run_bass_kernel_spmd

B = 2048
N = 50000
D = 128
K = 20
NCORES = 8
NL = N // NCORES            # 6250 codebook rows per core
NCHUNK = (NL + 127) // 128  # 49 transpose chunks
NP = NCHUNK * 128           # 6272 padded width
NCAND = 16                  # local candidates per row (2 max8 rounds); the
                            # host detects core/chunk exhaustion and falls back
NEG = -1.0e30

F32 = mybir.dt.float32
U32 = mybir.dt.uint32

# panel split of the 6272-wide similarity row: 12 x 512 + 1 x 128
PANELS = [(i * 512, 512) for i in range(12)] + [(12 * 512, 128)]


CHUNK = 392
NCH = 16  # 16 x 392 chunks of the 6272-wide row


def _topk_rounds(nc, x, mvals, midx, m1, nrounds=NCAND // 8):
    """Top (8*nrounds) values+indices per partition row of x [128, NP].

    Level 1: top-8 of each 256-chunk (m1 [128, 8*NCH]).  A chunk holding more
    than 8 of the row's top-24 loses the excess -- the host detects chunk
    exhaustion (8 finalists from one chunk) and recomputes those rows (~1e-4).
    Level 2: 3 max8/match_replace rounds on m1 give the top-24 values; their
    positions come from batched max_index scans of the untouched raw row.
    """
    for c in range(NCH):
        w = min(CHUNK, NP - c * CHUNK)
        nc.vector.max(out=m1[:, 8 * c : 8 * c + 8], in_=x[:, c * CHUNK : c * CHUNK + w])
    for r in range(nrounds):
        mv = mvals[:, 8 * r : 8 * r + 8]
        nc.vector.max(out=mv, in_=m1)
        if r + 1 < nrounds:
            nc.vector.match_replace(out=m1, in_to_replace=mv, in_values=m1, imm_value=NEG)
    for r in range(nrounds):
        nc.vector.max_index(
            out=midx[:, 8 * r : 8 * r + 8], in_max=mvals[:, 8 * r : 8 * r + 8], in_values=x
        )


def build_program():
    nc = bacc.Bacc("TRN2", target_bir_lowering=False, debug=False, num_devices=NCORES)

    zr = nc.dram_tensor("zr", [B, D], F32, kind="ExternalInput")
    zt = nc.dram_tensor("zt", [B, D], F32, kind="ExternalInput")
    rb = nc.dram_tensor("rb", [NP, D], F32, kind="ExternalInput")  # rot book slice
    tb = nc.dram_tensor("tb", [NP, D], F32, kind="ExternalInput")  # trans book slice

    cv = nc.dram_tensor("cv", [B, NCAND], F32, kind="ExternalOutput")
    ci = nc.dram_tensor("ci", [B, NCAND], U32, kind="ExternalOutput")
    ev = nc.dram_tensor("ev", [B, NCAND], F32, kind="ExternalOutput")
    ei = nc.dram_tensor("ei", [B, NCAND], U32, kind="ExternalOutput")

    bq_dram = nc.dram_tensor("bq_dram", [NP], F32)  # internal: -|x|^2/2 per entry

    with TileContext(nc) as tc:
        with (
            tc.tile_pool(name="persist", bufs=1) as persist,
            tc.tile_pool(name="ld", bufs=3) as ldpool,
            tc.tile_pool(name="tpsum", bufs=2, space="PSUM") as tpsum,
            tc.tile_pool(name="mmpsum", bufs=4, space="PSUM") as mmpsum,
            tc.tile_pool(name="s", bufs=2) as spool,
            tc.tile_pool(name="cand", bufs=2) as candpool,
            tc.tile_pool(name="small", bufs=2) as small,
        ):
            ident = persist.tile([128, 128], F32)
            make_identity(nc, ident)

            # ---- transpose queries: zT[d, b] for both query sets ----
            zrT = persist.tile([128, B], F32, tag="zrT")
            ztT = persist.tile([128, B], F32, tag="ztT")
            for src, dstT in ((zr, zrT), (zt, ztT)):
                for c in range(B // 128):
                    lt = ldpool.tile([128, D], F32, tag="qld")
                    nc.sync.dma_start(lt[:], src[ts(c, 128), :])
                    pt = tpsum.tile([128, 128], F32, tag="tp")
                    nc.tensor.transpose(pt, lt, ident)
                    nc.scalar.copy(dstT[:, ts(c, 128)], pt)

            # ---- transpose books: bookT[d, n]; trans book also -> bq ----
            rbT = persist.tile([128, NP], F32, tag="rbT")
            tbT = persist.tile([128, NP], F32, tag="tbT")
            bqcol = persist.tile([128, NCHUNK], F32, tag="bqcol")
            sqscr = persist.tile([128, D], F32, tag="sqscr")
            for src, dstT, want_bq in ((rb, rbT, False), (tb, tbT, True)):
                for c in range(NCHUNK):
                    lt = ldpool.tile([128, D], F32, tag="bld")
                    nc.sync.dma_start(lt[:], src[ts(c, 128), :])
                    if want_bq:
                        nc.scalar.activation(
                            sqscr[:],
                            lt[:],
                            mybir.ActivationFunctionType.Square,
                            accum_out=bqcol[:, c : c + 1],
                        )
                    pt = tpsum.tile([128, 128], F32, tag="tp")
                    nc.tensor.transpose(pt, lt, ident)
                    nc.scalar.copy(dstT[:, ts(c, 128)], pt)

            # bq -> -bq/2, reshape [128, 49] -> flat [6272] via DRAM, pad = NEG
            nc.scalar.mul(bqcol[:], bqcol[:], -0.5)
            bq_view = bq_dram.ap().rearrange("(c p) -> p c", p=128)  # [128, 49]
            nc.sync.dma_start(bq_view, bqcol[:])
            nbq = persist.tile([1, NP], F32, tag="nbq")
            nc.sync.dma_start(nbq[:], bq_dram.ap().rearrange("(a n) -> a n", a=1))
            if NP > NL:
                nc.vector.memset(nbq[:, NL:NP], NEG)

            ones1 = persist.tile([1, 128], F32, tag="ones1")
            nc.vector.memset(ones1[:], 1.0)

            # ---- main: per book, per 128-row query tile ----
            for book, (qT, bT) in enumerate(((zrT, rbT), (ztT, tbT))):
                for bt in range(B // 128):
                    s = spool.tile([128, NP], F32, tag="s")
                    for p0, pw in PANELS:
                        ps = mmpsum.tile([128, 512], F32, tag="mm")
                        nc.tensor.matmul(
                            ps[:, :pw],
                            qT[:, ts(bt, 128)],
                            bT[:, p0 : p0 + pw],
                            start=True,
                            stop=(book == 0),
                        )
                        if book == 1:
                            nc.tensor.matmul(
                                ps[:, :pw],
                                ones1[:],
                                nbq[:, p0 : p0 + pw],
                                start=False,
                                stop=True,
                            )
                        nc.scalar.copy(s[:, p0 : p0 + pw], ps[:, :pw])
                    if book == 0 and NP > NL:
                        nc.vector.memset(s[:, NL:NP], NEG)

                    mvals = candpool.tile([128, NCAND], F32, tag="mv")
                    midx = candpool.tile([128, NCAND], U32, tag="mi")
                    m1 = candpool.tile([128, 8 * NCH], F32, tag="m1")
                    _topk_rounds(nc, s, mvals, midx, m1)
                    vdst, idst = ((cv, ci), (ev, ei))[book]
                    nc.sync.dma_start(vdst[ts(bt, 128), :], mvals[:])
                    nc.sync.dma_start(idst[ts(bt, 128), :], midx[:])

    nc.compile()
    return nc


_NC = None


def _get_program():
    global _NC
    if _NC is None:
        _NC = build_program()
    return _NC


def kernel(z_rot, z_trans, z_rot_book, z_trans_book, rot_book, trans_book):
    z_rot = np.ascontiguousarray(z_rot, dtype=np.float32)
    z_trans = np.ascontiguousarray(z_trans, dtype=np.float32)
    z_rot_book = np.asarray(z_rot_book, dtype=np.float32)
    z_trans_book = np.asarray(z_trans_book, dtype=np.float32)

    nc = _get_program()

    in_maps = []
    for c in range(NCORES):
        sl = slice(c * NL, (c + 1) * NL)
        rbp = np.zeros((NP, D), np.float32)
        rbp[:NL] = z_rot_book[sl]
        tbp = np.zeros((NP, D), np.float32)
        tbp[:NL] = z_trans_book[sl]
        in_maps.append({"zr": z_rot, "zt": z_trans, "rb": rbp, "tb": tbp})

    global _last_in_maps
    _last_in_maps = in_maps
    res = run_bass_kernel_spmd(nc, in_maps, list(range(NCORES))).results

    # ---- host: gather candidates, merge to global top-20 ----
    cv = np.stack([res[c]["cv"] for c in range(NCORES)])  # [8, B, NCAND]
    ci = np.stack([res[c]["ci"] for c in range(NCORES)]).astype(np.int64)
    ev = np.stack([res[c]["ev"] for c in range(NCORES)])
    ei = np.stack([res[c]["ei"] for c in range(NCORES)]).astype(np.int64)

    offs = (np.arange(NCORES) * NL)[:, None, None]
    ci_g = np.transpose(ci + offs, (1, 0, 2)).reshape(B, -1)
    ei_g = np.transpose(ei + offs, (1, 0, 2)).reshape(B, -1)
    cv_g = np.transpose(cv, (1, 0, 2)).reshape(B, -1)
    ev_g = np.transpose(ev, (1, 0, 2)).reshape(B, -1)

    norm_r = np.sqrt(np.einsum("bd,bd->b", z_rot, z_rot))
    zq = np.einsum("bd,bd->b", z_trans, z_trans)

    cos_vals = cv_g / norm_r[:, None]          # true cosine values
    euc_vals = 2.0 * ev_g - zq[:, None]        # true m2 values

    def merge(vals, idx):
        # top-20 per row, descending, ties -> lowest index (matches lax.top_k)
        order = np.lexsort((idx, -vals), axis=1)[:, :K]
        r = np.arange(B)[:, None]
        return vals[r, order], idx[r, order]

    vals_cos, ind_cos = merge(cos_vals, ci_g)
    vals_euc, ind_euc = merge(euc_vals, ei_g)

    # safety fallback: rows with duplicate candidate indices (exact-value ties
    # can confuse max_index) get recomputed exactly on host.  ~never triggers.
    def fix_rows(bad, book, q, qscale, vals, ind, is_cos):
        for b in np.nonzero(bad)[0]:
            s = book.astype(np.float32) @ q[b].astype(np.float32)
            if is_cos:
                s = s / qscale[b]
            else:
                s = 2.0 * s - np.einsum("nd,nd->n", book, book) - qscale[b]
            o = np.lexsort((np.arange(N), -s))[:K]
            vals[b], ind[b] = s[o], o

    def suspicious(idx_g, ind_top):
        # duplicate candidate indices (exact-value ties through max_index)
        srt = np.sort(idx_g, axis=1)
        bad = (srt[:, 1:] == srt[:, :-1]).any(1)
        # chunk exhaustion: 8 of one core's candidates from one 392-chunk
        ch = np.sort((idx_g % NL) // CHUNK + (idx_g // NL) * 1000, axis=1)
        bad |= (ch[:, 7:] == ch[:, :-7]).any(1)
        # core exhaustion: one core supplies >= NCAND of the final top-20
        core = ind_top // NL
        hits = (core[:, :, None] == np.arange(NCORES)[None, None, :]).sum(1)
        bad |= hits.max(1) >= NCAND
        return bad

    bad_c = suspicious(ci_g, ind_cos)
    bad_e = suspicious(ei_g, ind_euc)
    if bad_c.any():
        fix_rows(bad_c, z_rot_book, z_rot, norm_r, vals_cos, ind_cos, True)
    if bad_e.any():
        fix_rows(bad_e, z_trans_book, z_trans, zq, vals_euc, ind_euc, False)

    ind_cos = ind_cos.astype(np.int32)
    ind_euc = ind_euc.astype(np.int32)
    labels_rot = np.asarray(rot_book)[ind_cos]      # [B, K, 1]
    labels_trans = np.asarray(trans_book)[ind_euc]  # [B, K, 3]

    return (
        vals_cos.astype(np.float32),
        ind_cos,
        labels_rot,
        vals_euc.astype(np.float32),
        ind_euc,
        labels_trans,
    )


# revision 12
# speedup vs baseline: 3452.9897x; 1.1778x over previous
"""Distributed kNN codebook kernel for Trainium2 (8 NeuronCores).

Sharding: codebook dimension N=50000 split 8 ways (6250 rows/core, padded to
6272).  Each core computes [2048, 6272] similarity panels on the PE, does a
local top-8 per query row on the DVE (one max8 + one batched max_index scan),
and writes 8 candidates/row.  Host merges 8*8 candidates to the global top-20
(falling back to an exact recompute for rows where one core supplied >= 8 of
the final 20) and gathers labels.

Selection math (order-preserving per row, exact values reconstructed on host):
  cos:    raw dot z_rot . book      (renorm of z is a positive row scale)
  euclid: s = z_trans . book - |book|^2/2   (m2 = 2*s - |z|^2)
"""

import sys

sys.path.insert(0, "/opt/trn_rl_repo")

import numpy as np

import concourse.bass as bass
import concourse.bacc as bacc
import concourse.mybir as mybir
from concourse.bass import ts
from concourse.masks import make_identity
from concourse.tile import TileContext
from concourse.bass_utils import run_bass_kernel_spmd

B = 2048
N = 50000
D = 128
K = 20
NCORES = 8
NL = N // NCORES            # 6250 codebook rows per core
NCHUNK = (NL + 127) // 128  # 49 transpose chunks
NP = NCHUNK * 128           # 6272 padded width
NCAND = 8                   # local candidates per row (one max8 round); the
                            # host detects core exhaustion and falls back
NEG = -1.0e30

F32 = mybir.dt.float32
U32 = mybir.dt.uint32

# panel split of the 6272-wide similarity row: 12 x 512 + 1 x 128
PANELS = [(i * 512, 512) for i in range(12)] + [(12 * 512, 128)]


def _top8(nc, x, mvals, midx):
    """Top-8 values+indices per partition row of x [128, NP]: one max8 scan
    for the values, one batched max_index scan for their positions."""
    nc.vector.max(out=mvals[:, :8], in_=x[:])
    nc.vector.max_index(out=midx[:, :8], in_max=mvals[:, :8], in_values=x[:])


def build_program():
    nc = bacc.Bacc("TRN2", target_bir_lowering=False, debug=False, num_devices=NCORES)

    zr = nc.dram_tensor("zr", [B, D], F32, kind="ExternalInput")
    zt = nc.dram_tensor("zt", [B, D], F32, kind="ExternalInput")
    rb = nc.dram_tensor("rb", [NP, D], F32, kind="ExternalInput")  # rot book slice
    tb = nc.dram_tensor("tb", [NP, D], F32, kind="ExternalInput")  # trans book slice

    cv = nc.dram_tensor("cv", [B, NCAND], F32, kind="ExternalOutput")
    ci = nc.dram_tensor("ci", [B, NCAND], U32, kind="ExternalOutput")
    ev = nc.dram_tensor("ev", [B, NCAND], F32, kind="ExternalOutput")
    ei = nc.dram_tensor("ei", [B, NCAND], U32, kind="ExternalOutput")

    bq_dram = nc.dram_tensor("bq_dram", [NP], F32)  # internal: -|x|^2/2 per entry

    with TileContext(nc) as tc:
        with (
            tc.tile_pool(name="persist", bufs=1) as persist,
            tc.tile_pool(name="ld", bufs=3) as ldpool,
            tc.tile_pool(name="tpsum", bufs=2, space="PSUM") as tpsum,
            tc.tile_pool(name="mmpsum", bufs=4, space="PSUM") as mmpsum,
            tc.tile_pool(name="s", bufs=2) as spool,
            tc.tile_pool(name="cand", bufs=2) as candpool,
            tc.tile_pool(name="small", bufs=2) as small,
        ):
            ident = persist.tile([128, 128], F32)
            make_identity(nc, ident)

            # ---- transpose queries: zT[d, b] for both query sets ----
            zrT = persist.tile([128, B], F32, tag="zrT")
            ztT = persist.tile([128, B], F32, tag="ztT")
            for src, dstT in ((zr, zrT), (zt, ztT)):
                for c in range(B // 128):
                    lt = ldpool.tile([128, D], F32, tag="qld")
                    nc.sync.dma_start(lt[:], src[ts(c, 128), :])
                    pt = tpsum.tile([128, 128], F32, tag="tp")
                    nc.tensor.transpose(pt, lt, ident)
                    nc.scalar.copy(dstT[:, ts(c, 128)], pt)

            # ---- transpose books: bookT[d, n]; trans book also -> bq ----
            rbT = persist.tile([128, NP], F32, tag="rbT")
            tbT = persist.tile([128, NP], F32, tag="tbT")
            bqcol = persist.tile([128, NCHUNK], F32, tag="bqcol")
            sqscr = persist.tile([128, D], F32, tag="sqscr")
            for src, dstT, want_bq in ((rb, rbT, False), (tb, tbT, True)):
                for c in range(NCHUNK):
                    lt = ldpool.tile([128, D], F32, tag="bld")
                    nc.sync.dma_start(lt[:], src[ts(c, 128), :])
                    if want_bq:
                        nc.scalar.activation(
                            sqscr[:],
                            lt[:],
                            mybir.ActivationFunctionType.Square,
                            accum_out=bqcol[:, c : c + 1],
                        )
                    pt = tpsum.tile([128, 128], F32, tag="tp")
                    nc.tensor.transpose(pt, lt, ident)
                    nc.scalar.copy(dstT[:, ts(c, 128)], pt)

            # bq -> -bq/2, reshape [128, 49] -> flat [6272] via DRAM, pad = NEG
            nc.scalar.mul(bqcol[:], bqcol[:], -0.5)
            bq_view = bq_dram.ap().rearrange("(c p) -> p c", p=128)  # [128, 49]
            nc.sync.dma_start(bq_view, bqcol[:])
            nbq = persist.tile([1, NP], F32, tag="nbq")
            nc.sync.dma_start(nbq[:], bq_dram.ap().rearrange("(a n) -> a n", a=1))
            if NP > NL:
                nc.vector.memset(nbq[:, NL:NP], NEG)

            ones1 = persist.tile([1, 128], F32, tag="ones1")
            nc.vector.memset(ones1[:], 1.0)

            # ---- main: per book, per 128-row query tile ----
            for book, (qT, bT) in enumerate(((zrT, rbT), (ztT, tbT))):
                for bt in range(B // 128):
                    s = spool.tile([128, NP], F32, tag="s")
                    for p0, pw in PANELS:
                        ps = mmpsum.tile([128, 512], F32, tag="mm")
                        nc.tensor.matmul(
                            ps[:, :pw],
                            qT[:, ts(bt, 128)],
                            bT[:, p0 : p0 + pw],
                            start=True,
                            stop=(book == 0),
                        )
                        if book == 1:
                            nc.tensor.matmul(
                                ps[:, :pw],
                                ones1[:],
                                nbq[:, p0 : p0 + pw],
                                start=False,
                                stop=True,
                            )
                        nc.scalar.copy(s[:, p0 : p0 + pw], ps[:, :pw])
                    if book == 0 and NP > NL:
                        nc.vector.memset(s[:, NL:NP], NEG)

                    mvals = candpool.tile([128, NCAND], F32, tag="mv")
                    midx = candpool.tile([128, NCAND], U32, tag="mi")
                    _top8(nc, s, mvals, midx)
                    vdst, idst = ((cv, ci), (ev, ei))[book]
                    nc.sync.dma_start(vdst[ts(bt, 128), :], mvals[:])
                    nc.sync.dma_start(idst[ts(bt, 128), :], midx[:])

    nc.compile()
    return nc


_NC = None


def _get_program():
    global _NC
    if _NC is None:
        _NC = build_program()
    return _NC


def kernel(z_rot, z_trans, z_rot_book, z_trans_book, rot_book, trans_book):
    z_rot = np.ascontiguousarray(z_rot, dtype=np.float32)
    z_trans = np.ascontiguousarray(z_trans, dtype=np.float32)
    z_rot_book = np.asarray(z_rot_book, dtype=np.float32)
    z_trans_book = np.asarray(z_trans_book, dtype=np.float32)

    nc = _get_program()

    in_maps = []
    for c in range(NCORES):
        sl = slice(c * NL, (c + 1) * NL)
        rbp = np.zeros((NP, D), np.float32)
        rbp[:NL] = z_rot_book[sl]
        tbp = np.zeros((NP, D), np.float32)
        tbp[:NL] = z_trans_book[sl]
        in_maps.append({"zr": z_rot, "zt": z_trans, "rb": rbp, "tb": tbp})

    global _last_in_maps
    _last_in_maps = in_maps
    res = run_bass_kernel_spmd(nc, in_maps, list(range(NCORES))).results

    # ---- host: gather candidates, merge to global top-20 ----
    cv = np.stack([res[c]["cv"] for c in range(NCORES)])  # [8, B, NCAND]
    ci = np.stack([res[c]["ci"] for c in range(NCORES)]).astype(np.int64)
    ev = np.stack([res[c]["ev"] for c in range(NCORES)])
    ei = np.stack([res[c]["ei"] for c in range(NCORES)]).astype(np.int64)

    offs = (np.arange(NCORES) * NL)[:, None, None]
    ci_g = np.transpose(ci + offs, (1, 0, 2)).reshape(B, -1)
    ei_g = np.transpose(ei + offs, (1, 0, 2)).reshape(B, -1)
    cv_g = np.transpose(cv, (1, 0, 2)).reshape(B, -1)
    ev_g = np.transpose(ev, (1, 0, 2)).reshape(B, -1)

    norm_r = np.sqrt(np.einsum("bd,bd->b", z_rot, z_rot))
    zq = np.einsum("bd,bd->b", z_trans, z_trans)

    cos_vals = cv_g / norm_r[:, None]          # true cosine values
    euc_vals = 2.0 * ev_g - zq[:, None]        # true m2 values

    def merge(vals, idx):
        # top-20 per row, descending, ties -> lowest index (matches lax.top_k)
        order = np.lexsort((idx, -vals), axis=1)[:, :K]
        r = np.arange(B)[:, None]
        return vals[r, order], idx[r, order]

    vals_cos, ind_cos = merge(cos_vals, ci_g)
    vals_euc, ind_euc = merge(euc_vals, ei_g)

    # safety fallback: rows with duplicate candidate indices (exact-value ties
    # can confuse max_index) get recomputed exactly on host.  ~never triggers.
    def fix_rows(bad, book, q, qscale, vals, ind, is_cos):
        for b in np.nonzero(bad)[0]:
            s = book.astype(np.float32) @ q[b].astype(np.float32)
            if is_cos:
                s = s / qscale[b]
            else:
                s = 2.0 * s - np.einsum("nd,nd->n", book, book) - qscale[b]
            o = np.lexsort((np.arange(N), -s))[:K]
            vals[b], ind[b] = s[o], o

    def suspicious(idx_g, ind_top):
        # duplicate candidate indices (exact-value ties through max_index)
        srt = np.sort(idx_g, axis=1)
        bad = (srt[:, 1:] == srt[:, :-1]).any(1)
        # core exhaustion: one core supplies >= NCAND of the final top-20,
        # so its unreturned 9th-best might also belong in the top-20
        core = ind_top // NL
        hits = (core[:, :, None] == np.arange(NCORES)[None, None, :]).sum(1)
        bad |= hits.max(1) >= NCAND
        return bad

    bad_c = suspicious(ci_g, ind_cos)
    bad_e = suspicious(ei_g, ind_euc)
    if bad_c.any():
        fix_rows(bad_c, z_rot_book, z_rot, norm_r, vals_cos, ind_cos, True)
    if bad_e.any():
        fix_rows(bad_e, z_trans_book, z_trans, zq, vals_euc, ind_euc, False)

    ind_cos = ind_cos.astype(np.int32)
    ind_euc = ind_euc.astype(np.int32)
    labels_rot = np.asarray(rot_book)[ind_cos]      # [B, K, 1]
    labels_trans = np.asarray(trans_book)[ind_euc]  # [B, K, 3]

    return (
        vals_cos.astype(np.float32),
        ind_cos,
        labels_rot,
        vals_euc.astype(np.float32),
        ind_euc,
        labels_trans,
    )
